# revision 18
# baseline (speedup 1.0000x reference)
"""Multi-head attention (B=8, S=1024, D=768, H=12) on 8 TRN2 NeuronCores.

Sharding: pure batch parallelism - one batch element per core, weights
replicated. No collectives needed.

v2: engine-rebalanced pipeline. The key structural change vs v1 is that all
softmax-weight transposes ([q,k] -> [k,q]) and the attn transpose run on the
DMA XBAR (dma_start_transpose, 16x128 tiles) instead of PE+PSUM staging +
ACT evacuation. The XBAR's blocked 3D output (out[p, j, q] = in[q, j*128+p])
is exactly the per-128-chunk transposed layout the AV / projection matmuls
need as stationary operands.

Per-core pipeline (tokens T=1024, D=768, H=12 heads of HD=64):
  1. x loaded and PE-transposed to xT (fp32 -> fp32r via ACT evacuation);
     W_qkv DMA'd fp32 and rounded to fp32r by GPSIMD copies (V columns
     first); W_proj SWDGE-cast to bf16.
  2. V [T, 768] (bf16, with a ones column per head whose AV output column is
     the softmax denominator); Q^T,K^T [768, T] kept fp32r, Q pre-scaled x8
     during ACT evacuation so softmax needs no separate scale pass.
  3. Per head h, per query chunk qi (causal k <= (qi+1)*128):
       s = Q_h K_h^T (fp32r)         [PE]
       diagonal block mask add       [DVE]
       m = rowmax(s), negated        [DVE]
       w = exp(s - m) -> bf16        [ACT]
       wT = XBAR transpose of w      [DMA, issued from SP]
       o|Z = w @ [V_h | 1]           [PE, bf16]
       attn[:, h*64:] = o * (1/Z)    [recip DVE, scale copy ACT]
     QK chunk pairs are software-pipelined with the heads that consume them.
  4. In the last head's sweep each finished token chunk is XBAR-transposed
     and projected: y = attn @ W_proj + b, then DMA'd out.

Measured vs the fp32 jax reference: rel err ~3e-3 (scores and softmax stats
in fp32/tf32; only w/V/attn/W_proj are bf16).
"""

import numpy as np

import concourse.bass as bass
import concourse.mybir as mybir
import concourse.tile as tile
from concourse import bacc
from concourse.bass_utils import run_bass_kernel_spmd
from concourse.masks import make_causal_mask, make_identity

B, S, D = 8, 1024, 768
H, HD = 12, 64
HV = 65  # V block width per head: 64 value cols + the ones column
NT = S // 128   # 8 token chunks
ND = D // 128   # 6 d chunks
F32 = mybir.dt.float32
F32R = mybir.dt.float32r
BF16 = mybir.dt.bfloat16

N_CORES = 8


def bank_chunks(size):
    """Split [0, size) into matmul-N chunks that each sit in one PSUM bank
    (fp32 bank = 512 elems) and are >=256 where possible (fp32r full rate)."""
    out = []
    start = 0
    while start < size:
        end = min(start + 512, size, (start // 512 + 1) * 512)
        out.append((start, end))
        start = end
    return out


def build_mha(nc):
    x_d = nc.dram_tensor("x", [S, D], F32, kind="ExternalInput")
    wqkv_d = nc.dram_tensor("W_qkv", [D, 3 * D], F32, kind="ExternalInput")
    wproj_d = nc.dram_tensor("W_proj", [D, D], F32, kind="ExternalInput")
    bproj_d = nc.dram_tensor("b_proj", [1, D], F32, kind="ExternalInput")
    out_d = nc.dram_tensor("out", [S, D], F32, kind="ExternalOutput")

    with tile.TileContext(nc) as tc:
        with (
            tc.tile_pool(name="persist", bufs=1) as pp,
            tc.tile_pool(name="psum", bufs=1, space="PSUM") as psum,
        ):
            def pring():
                # big ring: score tiles + V/QK/output projections + startup
                # staging (3 x 2 banks)
                return psum.tile([128, 1024], F32, name="p1", tag="pring", bufs=3)

            def po_tile():
                # AV outputs + startup staging ring
                return psum.tile([128, 512], F32, name="pt1", tag="ptiny", bufs=2)

            def pwide():
                return pring()

            # ---- constants ----
            ident_f32 = pp.tile([128, 128], F32, name="ident_f32", tag="ident_f32")
            make_identity(nc, ident_f32[:])
            # causal mask for the diagonal 128x128 block
            bigmask = pp.tile([128, 128], F32, name="bigmask", tag="bigmask")
            make_causal_mask(nc, bigmask[:], mask_val=-1e10)

            # b_proj broadcast to 128 partitions via K=1 outer product
            b_row = pp.tile([1, D], F32, name="b_row", tag="b_row")
            nc.sync.dma_start(b_row[:], bproj_d[:])
            ones_col = pp.tile([1, 128], F32, name="ones_col", tag="ones_col")
            nc.vector.memset(ones_col[:], 1.0)
            b_bcast = pp.tile([128, D], F32, name="b_bcast", tag="b_bcast")
            pb = po_tile()
            for c0, c1 in bank_chunks(D):
                nc.tensor.matmul(
                    pb[:, 0:c1 - c0], ones_col[:], b_row[:, c0:c1],
                    start=True, stop=True,
                )
                nc.vector.tensor_copy(b_bcast[:, c0:c1], pb[:, 0:c1 - c0])

            # ---- persistent activations ----
            qkT = [pp.tile([128, S], F32R, name=f"qkT{m}", tag=f"qkT{m}") for m in range(12)]
            v_sb = [pp.tile([128, H * HV], BF16, name=f"v{qi}", tag=f"v{qi}") for qi in range(NT)]
            attn = [pp.tile([128, D], BF16, name=f"attn{qi}", tag=f"attn{qi}") for qi in range(NT)]
            wq = [pp.tile([128, 3 * D], F32R, name=f"wq{di}", tag=f"wq{di}") for di in range(ND)]
            wp = [pp.tile([128, D], BF16, name=f"wp{di}", tag=f"wp{di}") for di in range(ND)]

            with (
                tc.tile_pool(name="xpool", bufs=2) as xp,
                tc.tile_pool(name="wqstage", bufs=2) as wqs,
                tc.tile_pool(name="xtp", bufs=1) as xtp,
                tc.tile_pool(name="softmax", bufs=3) as p3s,
                tc.tile_pool(name="wtp", bufs=3) as wtp,
                tc.tile_pool(name="attp", bufs=2) as a3p,
                tc.tile_pool(name="ypool", bufs=2) as yp,
            ):
                # ---- loads: x (SP) + W_qkv V columns (ACT) first so the V
                # projection can start early; Q/K columns follow on SP so
                # they never delay x or the ACT-side xT evacuations.
                # GPSIMD copies do the fp32 -> tf32 rounding. ----
                def wq_load(part, di, eng):
                    wq_stage = wqs.tile([128, D], F32, name="wq_stage", tag="wq_stage")
                    eng.dma_start(
                        wq_stage[:],
                        wqkv_d[di * 128:(di + 1) * 128,
                               part * D:(part + 1) * D],
                    )
                    nc.gpsimd.tensor_copy(
                        wq[di][:, part * D:(part + 1) * D], wq_stage[:]
                    )

                for di in range(ND):
                    wq_load(2, di, nc.scalar)

                # ---- x load + transpose (fp32 PE transpose, ACT evacuation
                # rounds to fp32r) ----
                xT = [xtp.tile([128, S], F32R, name=f"xT{di}", tag=f"xT{di}") for di in range(ND)]
                for qi in range(NT):
                    x_t = xp.tile([128, D], F32, name="x_t", tag="x_t")
                    nc.sync.dma_start(x_t[:], x_d[qi * 128:(qi + 1) * 128, :])
                    for di0 in range(0, ND, 4):
                        nb = min(4, ND - di0)
                        pt = po_tile()
                        for j in range(nb):
                            nc.tensor.transpose(
                                pt[:, j * 128:(j + 1) * 128],
                                x_t[:, (di0 + j) * 128:(di0 + j + 1) * 128],
                                ident_f32[:],
                            )
                        for j in range(nb):
                            nc.scalar.copy(
                                xT[di0 + j][:, qi * 128:(qi + 1) * 128],
                                pt[:, j * 128:(j + 1) * 128],
                            )

                for part in (0, 1):
                    for di in range(ND):
                        wq_load(part, di, nc.sync)

                # ---- V in [token, dv] layout, with ones column per head ----
                for qi in range(NT):
                    pv = pwide()
                    for c0, c1 in bank_chunks(D):
                        for di in range(ND):
                            nc.tensor.matmul(
                                pv[:, c0:c1],
                                xT[di][:, qi * 128:(qi + 1) * 128],
                                wq[di][:, 2 * D + c0:2 * D + c1],
                                start=(di == 0), stop=(di == ND - 1),
                            )
                    nc.gpsimd.memset(
                        v_sb[qi][:].rearrange("p (h v) -> p h v", v=HV)[:, :, HD:], 1.0
                    )
                    nc.vector.tensor_copy(
                        v_sb[qi][:].rearrange("p (h v) -> p h v", v=HV)[:, :, :HD],
                        pv[:, :D].rearrange("p (h v) -> p h v", v=HD),
                    )

                for di in range(ND):
                    # SWDGE cast fp32 -> bf16 during load; issued late so the
                    # Pool engine rounds the W_qkv tiles first
                    nc.gpsimd.dma_start(
                        wp[di][:], wproj_d[di * 128:(di + 1) * 128, :]
                    )

                def qk_chunk(m):
                    pqk = pwide()
                    for c0, c1 in bank_chunks(S):
                        for di in range(ND):
                            nc.tensor.matmul(
                                pqk[:, c0:c1],
                                wq[di][:, m * 128:(m + 1) * 128],
                                xT[di][:, c0:c1],
                                start=(di == 0), stop=(di == ND - 1),
                            )
                    if m < 6:
                        # pre-scale Q by 8: scores then come out as 8*s and
                        # the softmax needs no separate x8 pass
                        nc.scalar.mul(qkT[m][:], pqk[:], 8.0)
                    else:
                        nc.scalar.copy(qkT[m][:], pqk[:])

                def attn_front(h, qi):
                    """scores -> mask -> rowmax -> exp -> XBAR transpose
                    launch. Returns the in-flight wT tile for attn_back."""
                    qoff = (h % 2) * 64
                    Qt = qkT[h // 2]
                    Kt = qkT[6 + h // 2]
                    ks = (qi + 1) * 128
                    lhs = Qt[qoff:qoff + 64, qi * 128:(qi + 1) * 128]
                    ps = pring()
                    for c0, c1 in bank_chunks(ks):
                        nc.tensor.matmul(
                            ps[:, c0:c1],
                            lhs,
                            Kt[qoff:qoff + 64, c0:c1],
                            start=True, stop=True,
                        )
                    # causal mask on the diagonal block
                    nc.vector.tensor_tensor(
                        out=ps[:, qi * 128:ks],
                        in0=ps[:, qi * 128:ks],
                        in1=bigmask[:],
                        op=mybir.AluOpType.add,
                    )
                    neg8m = p3s.tile([128, 1], F32, name="neg8m", tag="neg8m", bufs=8)
                    nc.vector.reduce_max(
                        out=neg8m[:], in_=ps[:, :ks],
                        axis=mybir.AxisListType.X, negate=True,
                    )
                    w_t = p3s.tile([128, S], BF16, name="w_t", tag="w_t", bufs=4)
                    nc.scalar.activation(
                        w_t[:, :ks], ps[:, :ks],
                        mybir.ActivationFunctionType.Exp,
                        bias=neg8m[:], scale=1.0,
                    )
                    # w[q, k] -> wT[k in chunk, ki, q] on the DMA XBAR;
                    # blocked output: wT[p, ki, q] = w[q, ki*128+p]
                    wT = wtp.tile([128, NT, 128], BF16, name="wT", tag="wT", bufs=5)
                    nc.sync.dma_start(
                        wT[:, :qi + 1, :], w_t[:, :ks], transpose=True
                    )
                    return wT

                def attn_av(h, qi, wT):
                    """AV matmul off the landed transpose."""
                    # o = w @ [V_h | 1]; last column = softmax denominator
                    po = po_tile()
                    for ki in range(qi + 1):
                        nc.tensor.matmul(
                            po[:, :HV],
                            wT[:, ki, :],
                            v_sb[ki][:, h * HV:(h + 1) * HV],
                            start=(ki == 0), stop=(ki == qi),
                        )
                    return po

                def attn_fin(h, qi, po):
                    """normalize: runs two blocks after its AV so the DVE/ACT
                    queue heads never wait on a fresh PE result."""
                    recip = p3s.tile([128, 1], F32, name="recip", tag="recip", bufs=8)
                    nc.vector.reciprocal(recip[:], po[:, HD:HV])
                    nc.scalar.mul(
                        attn[qi][:, h * HD:(h + 1) * HD],
                        po[:, :HD],
                        recip[:],
                    )

                def proj_front(qi):
                    """all heads done for token chunk qi: launch its XBAR
                    transpose. att3[p, di, q] = attn[q, di*128+p]"""
                    att3 = a3p.tile([128, ND, 128], BF16, name="att3", tag="att3", bufs=3)
                    nc.sync.dma_start(att3[:], attn[qi][:], transpose=True)
                    return att3

                def proj_back(qi, att3):
                    y_t = yp.tile([128, D], F32, name="y_t", tag="y_t")
                    py = pwide()
                    for c0, c1 in bank_chunks(D):
                        for di in range(ND):
                            nc.tensor.matmul(
                                py[:, c0:c1],
                                att3[:, di, :],
                                wp[di][:, c0:c1],
                                start=(di == 0), stop=(di == ND - 1),
                            )
                    nc.vector.tensor_tensor(
                        out=y_t[:], in0=py[:, :D], in1=b_bcast[:],
                        op=mybir.AluOpType.add,
                    )
                    # SWDGE store keeps the (wait-blocking) HWDGE
                    # sequencers free for the transpose stream
                    nc.gpsimd.dma_start(
                        out_d[qi * 128:(qi + 1) * 128, :], y_t[:]
                    )

                # software pipeline: each block's AV runs LAG_AV blocks after
                # its XBAR transpose was launched (hiding the ~2.5us DMA
                # latency behind other blocks' scores/exp), and its normalize
                # a further LAG_FIN-LAG_AV blocks later (so DVE/ACT queue
                # heads never wait on fresh PE results). The last head's
                # finished token chunks go through the same lagging for their
                # projection. QK chunk pairs for the next head pair are issued
                # mid-head to spread PSUM-ring pressure.
                LAG_AV = 4
                LAG_FIN = 5
                front_q = []
                av_q = []
                proj_q = []

                def drain(front_limit, av_limit):
                    while len(front_q) > front_limit:
                        h, qi, wT = front_q.pop(0)
                        av_q.append((h, qi, attn_av(h, qi, wT)))
                    while len(av_q) > av_limit:
                        h, qi, po = av_q.pop(0)
                        attn_fin(h, qi, po)
                        if h == H - 1:
                            proj_q.append((qi, proj_front(qi)))
                        if len(proj_q) > 2:
                            proj_back(*proj_q.pop(0))

                qk_chunk(0)
                qk_chunk(6)
                for h in range(H):
                    r = h // 2
                    for qi in range(NT):
                        front_q.append((h, qi, attn_front(h, qi)))
                        drain(LAG_AV, LAG_FIN - LAG_AV)
                        if h % 2 == 1 and r < 5:
                            if qi == 2:
                                qk_chunk(r + 1)
                            elif qi == 5:
                                qk_chunk(7 + r)
                drain(0, 0)
                while proj_q:
                    proj_back(*proj_q.pop(0))

    nc.compile()
    return nc


_NC_CACHE = None


def _get_nc():
    global _NC_CACHE
    if _NC_CACHE is None:
        nc = bacc.Bacc(
            "TRN2",
            target_bir_lowering=False,
            debug=False,
            num_devices=N_CORES,
        )
        build_mha(nc)
        _NC_CACHE = nc
    return _NC_CACHE


def kernel(x, W_qkv, W_proj, b_proj):
    nc = _get_nc()
    x = np.ascontiguousarray(np.asarray(x, dtype=np.float32))
    W_qkv = np.ascontiguousarray(np.asarray(W_qkv, dtype=np.float32))
    W_proj = np.ascontiguousarray(np.asarray(W_proj, dtype=np.float32))
    b_proj = np.ascontiguousarray(
        np.asarray(b_proj, dtype=np.float32).reshape(1, D)
    )
    in_maps = [
        {"x": x[b], "W_qkv": W_qkv, "W_proj": W_proj, "b_proj": b_proj}
        for b in range(N_CORES)
    ]
    res = run_bass_kernel_spmd(nc, in_maps, core_ids=list(range(N_CORES)))
    return np.stack([res.results[b]["out"] for b in range(N_CORES)], axis=0)


# revision 19
# speedup vs baseline: 1.0040x; 1.0040x over previous
"""Multi-head attention (B=8, S=1024, D=768, H=12) on 8 TRN2 NeuronCores.

Sharding: pure batch parallelism - one batch element per core, weights
replicated. No collectives needed.

v2: engine-rebalanced pipeline. The key structural change vs v1 is that all
softmax-weight transposes ([q,k] -> [k,q]) and the attn transpose run on the
DMA XBAR (dma_start_transpose, 16x128 tiles) instead of PE+PSUM staging +
ACT evacuation. The XBAR's blocked 3D output (out[p, j, q] = in[q, j*128+p])
is exactly the per-128-chunk transposed layout the AV / projection matmuls
need as stationary operands.

Per-core pipeline (tokens T=1024, D=768, H=12 heads of HD=64):
  1. x loaded and PE-transposed to xT (fp32 -> fp32r via ACT evacuation);
     W_qkv DMA'd fp32 and rounded to fp32r by GPSIMD copies (V columns
     first); W_proj SWDGE-cast to bf16.
  2. V [T, 768] (bf16, with a ones column per head whose AV output column is
     the softmax denominator); Q^T,K^T [768, T] kept fp32r, Q pre-scaled x8
     during ACT evacuation so softmax needs no separate scale pass.
  3. Per head h, per query chunk qi (causal k <= (qi+1)*128):
       s = Q_h K_h^T (fp32r)         [PE]
       diagonal block mask add       [DVE]
       m = rowmax(s), negated        [DVE]
       w = exp(s - m) -> bf16        [ACT]
       wT = XBAR transpose of w      [DMA, issued from SP]
       o|Z = w @ [V_h | 1]           [PE, bf16]
       attn[:, h*64:] = o * (1/Z)    [recip DVE, scale copy ACT]
     QK chunk pairs are software-pipelined with the heads that consume them.
  4. In the last head's sweep each finished token chunk is XBAR-transposed
     and projected: y = attn @ W_proj + b, then DMA'd out.

Measured vs the fp32 jax reference: rel err ~3e-3 (scores and softmax stats
in fp32/tf32; only w/V/attn/W_proj are bf16).
"""

import numpy as np

import concourse.bass as bass
import concourse.mybir as mybir
import concourse.tile as tile
from concourse import bacc
from concourse.bass_utils import run_bass_kernel_spmd
from concourse.masks import make_causal_mask, make_identity

B, S, D = 8, 1024, 768
H, HD = 12, 64
HV = 65  # V block width per head: 64 value cols + the ones column
NT = S // 128   # 8 token chunks
ND = D // 128   # 6 d chunks
F32 = mybir.dt.float32
F32R = mybir.dt.float32r
BF16 = mybir.dt.bfloat16

N_CORES = 8


def bank_chunks(size):
    """Split [0, size) into matmul-N chunks that each sit in one PSUM bank
    (fp32 bank = 512 elems) and are >=256 where possible (fp32r full rate)."""
    out = []
    start = 0
    while start < size:
        end = min(start + 512, size, (start // 512 + 1) * 512)
        out.append((start, end))
        start = end
    return out


def build_mha(nc):
    x_d = nc.dram_tensor("x", [S, D], F32, kind="ExternalInput")
    wqkv_d = nc.dram_tensor("W_qkv", [D, 3 * D], F32, kind="ExternalInput")
    wproj_d = nc.dram_tensor("W_proj", [D, D], F32, kind="ExternalInput")
    bproj_d = nc.dram_tensor("b_proj", [1, D], F32, kind="ExternalInput")
    out_d = nc.dram_tensor("out", [S, D], F32, kind="ExternalOutput")

    with tile.TileContext(nc) as tc:
        with (
            tc.tile_pool(name="persist", bufs=1) as pp,
            tc.tile_pool(name="psum", bufs=1, space="PSUM") as psum,
        ):
            def pring():
                # big ring: score tiles + V/QK/output projections + startup
                # staging (3 x 2 banks)
                return psum.tile([128, 1024], F32, name="p1", tag="pring", bufs=3)

            def po_tile():
                # AV outputs + startup staging ring
                return psum.tile([128, 512], F32, name="pt1", tag="ptiny", bufs=2)

            def pwide():
                return pring()

            # ---- constants ----
            ident_f32 = pp.tile([128, 128], F32, name="ident_f32", tag="ident_f32")
            make_identity(nc, ident_f32[:])
            # causal mask for the diagonal 128x128 block
            bigmask = pp.tile([128, 128], F32, name="bigmask", tag="bigmask")
            make_causal_mask(nc, bigmask[:], mask_val=-1e10)

            # b_proj broadcast to 128 partitions via K=1 outer product
            b_row = pp.tile([1, D], F32, name="b_row", tag="b_row")
            nc.sync.dma_start(b_row[:], bproj_d[:])
            ones_col = pp.tile([1, 128], F32, name="ones_col", tag="ones_col")
            nc.vector.memset(ones_col[:], 1.0)
            b_bcast = pp.tile([128, D], F32, name="b_bcast", tag="b_bcast")
            pb = po_tile()
            for c0, c1 in bank_chunks(D):
                nc.tensor.matmul(
                    pb[:, 0:c1 - c0], ones_col[:], b_row[:, c0:c1],
                    start=True, stop=True,
                )
                nc.vector.tensor_copy(b_bcast[:, c0:c1], pb[:, 0:c1 - c0])

            # ---- persistent activations ----
            qkT = [pp.tile([128, S], F32R, name=f"qkT{m}", tag=f"qkT{m}") for m in range(12)]
            v_sb = [pp.tile([128, H * HV], BF16, name=f"v{qi}", tag=f"v{qi}") for qi in range(NT)]
            attn = [pp.tile([128, D], BF16, name=f"attn{qi}", tag=f"attn{qi}") for qi in range(NT)]
            wq = [pp.tile([128, 3 * D], F32R, name=f"wq{di}", tag=f"wq{di}") for di in range(ND)]
            wp = [pp.tile([128, D], BF16, name=f"wp{di}", tag=f"wp{di}") for di in range(ND)]

            with (
                tc.tile_pool(name="xpool", bufs=2) as xp,
                tc.tile_pool(name="wqstage", bufs=2) as wqs,
                tc.tile_pool(name="xtp", bufs=1) as xtp,
                tc.tile_pool(name="softmax", bufs=3) as p3s,
                tc.tile_pool(name="wtp", bufs=3) as wtp,
                tc.tile_pool(name="attp", bufs=2) as a3p,
                tc.tile_pool(name="ypool", bufs=2) as yp,
            ):
                # ---- loads: x (SP) + W_qkv V columns (ACT) first so the V
                # projection can start early; Q/K columns follow on SP so
                # they never delay x or the ACT-side xT evacuations.
                # GPSIMD copies do the fp32 -> tf32 rounding. ----
                def wq_load(part, di, eng):
                    wq_stage = wqs.tile([128, D], F32, name="wq_stage", tag="wq_stage")
                    eng.dma_start(
                        wq_stage[:],
                        wqkv_d[di * 128:(di + 1) * 128,
                               part * D:(part + 1) * D],
                    )
                    nc.gpsimd.tensor_copy(
                        wq[di][:, part * D:(part + 1) * D], wq_stage[:]
                    )

                for di in range(ND):
                    wq_load(2, di, nc.scalar)

                # ---- x load + transpose (fp32 PE transpose, ACT evacuation
                # rounds to fp32r) ----
                xT = [xtp.tile([128, S], F32R, name=f"xT{di}", tag=f"xT{di}") for di in range(ND)]
                for qi in range(NT):
                    x_t = xp.tile([128, D], F32, name="x_t", tag="x_t")
                    nc.sync.dma_start(x_t[:], x_d[qi * 128:(qi + 1) * 128, :])
                    for di0 in range(0, ND, 4):
                        nb = min(4, ND - di0)
                        pt = po_tile()
                        for j in range(nb):
                            nc.tensor.transpose(
                                pt[:, j * 128:(j + 1) * 128],
                                x_t[:, (di0 + j) * 128:(di0 + j + 1) * 128],
                                ident_f32[:],
                            )
                        for j in range(nb):
                            nc.scalar.copy(
                                xT[di0 + j][:, qi * 128:(qi + 1) * 128],
                                pt[:, j * 128:(j + 1) * 128],
                            )

                for part in (0, 1):
                    for di in range(ND):
                        wq_load(part, di, nc.sync)

                # ---- V in [token, dv] layout, with ones column per head ----
                for qi in range(NT):
                    pv = pwide()
                    for c0, c1 in bank_chunks(D):
                        for di in range(ND):
                            nc.tensor.matmul(
                                pv[:, c0:c1],
                                xT[di][:, qi * 128:(qi + 1) * 128],
                                wq[di][:, 2 * D + c0:2 * D + c1],
                                start=(di == 0), stop=(di == ND - 1),
                            )
                    nc.gpsimd.memset(
                        v_sb[qi][:].rearrange("p (h v) -> p h v", v=HV)[:, :, HD:], 1.0
                    )
                    nc.vector.tensor_copy(
                        v_sb[qi][:].rearrange("p (h v) -> p h v", v=HV)[:, :, :HD],
                        pv[:, :D].rearrange("p (h v) -> p h v", v=HD),
                    )

                for di in range(ND):
                    # SWDGE cast fp32 -> bf16 during load; issued late so the
                    # Pool engine rounds the W_qkv tiles first
                    nc.gpsimd.dma_start(
                        wp[di][:], wproj_d[di * 128:(di + 1) * 128, :]
                    )

                def qk_chunk(m):
                    pqk = pwide()
                    for c0, c1 in bank_chunks(S):
                        for di in range(ND):
                            nc.tensor.matmul(
                                pqk[:, c0:c1],
                                wq[di][:, m * 128:(m + 1) * 128],
                                xT[di][:, c0:c1],
                                start=(di == 0), stop=(di == ND - 1),
                            )
                    if m < 6:
                        # pre-scale Q by 8: scores then come out as 8*s and
                        # the softmax needs no separate x8 pass
                        nc.scalar.mul(qkT[m][:], pqk[:], 8.0)
                    else:
                        nc.scalar.copy(qkT[m][:], pqk[:])

                def attn_front(h, qi):
                    """scores -> mask -> rowmax -> exp -> XBAR transpose
                    launch. Returns the in-flight wT tile for attn_back."""
                    qoff = (h % 2) * 64
                    Qt = qkT[h // 2]
                    Kt = qkT[6 + h // 2]
                    ks = (qi + 1) * 128
                    lhs = Qt[qoff:qoff + 64, qi * 128:(qi + 1) * 128]
                    ps = pring()
                    for c0, c1 in bank_chunks(ks):
                        nc.tensor.matmul(
                            ps[:, c0:c1],
                            lhs,
                            Kt[qoff:qoff + 64, c0:c1],
                            start=True, stop=True,
                        )
                    # causal mask on the diagonal block
                    nc.vector.tensor_tensor(
                        out=ps[:, qi * 128:ks],
                        in0=ps[:, qi * 128:ks],
                        in1=bigmask[:],
                        op=mybir.AluOpType.add,
                    )
                    neg8m = p3s.tile([128, 1], F32, name="neg8m", tag="neg8m", bufs=8)
                    nc.vector.reduce_max(
                        out=neg8m[:], in_=ps[:, :ks],
                        axis=mybir.AxisListType.X, negate=True,
                    )
                    w_t = p3s.tile([128, S], BF16, name="w_t", tag="w_t", bufs=4)
                    nc.scalar.activation(
                        w_t[:, :ks], ps[:, :ks],
                        mybir.ActivationFunctionType.Exp,
                        bias=neg8m[:], scale=1.0,
                    )
                    # w[q, k] -> wT[k in chunk, ki, q] on the DMA XBAR;
                    # blocked output: wT[p, ki, q] = w[q, ki*128+p].
                    # Per-qi tags: the same slot is reused only 8 blocks
                    # later (by the next head), a deeper effective ring than
                    # fixed-size buffers could afford in SBUF.
                    wT = wtp.tile([128, qi + 1, 128], BF16, name="wT",
                                  tag=f"wT{qi}", bufs=1)
                    nc.sync.dma_start(wT[:], w_t[:, :ks], transpose=True)
                    return wT

                def attn_av(h, qi, wT):
                    """AV matmul off the landed transpose."""
                    # o = w @ [V_h | 1]; last column = softmax denominator
                    po = po_tile()
                    for ki in range(qi + 1):
                        nc.tensor.matmul(
                            po[:, :HV],
                            wT[:, ki, :],
                            v_sb[ki][:, h * HV:(h + 1) * HV],
                            start=(ki == 0), stop=(ki == qi),
                        )
                    return po

                def attn_fin(h, qi, po):
                    """normalize: runs two blocks after its AV so the DVE/ACT
                    queue heads never wait on a fresh PE result."""
                    recip = p3s.tile([128, 1], F32, name="recip", tag="recip", bufs=8)
                    nc.vector.reciprocal(recip[:], po[:, HD:HV])
                    nc.scalar.mul(
                        attn[qi][:, h * HD:(h + 1) * HD],
                        po[:, :HD],
                        recip[:],
                    )

                def proj_front(qi):
                    """all heads done for token chunk qi: launch its XBAR
                    transpose. att3[p, di, q] = attn[q, di*128+p]"""
                    att3 = a3p.tile([128, ND, 128], BF16, name="att3", tag="att3", bufs=3)
                    nc.sync.dma_start(att3[:], attn[qi][:], transpose=True)
                    return att3

                def proj_back(qi, att3):
                    y_t = yp.tile([128, D], F32, name="y_t", tag="y_t")
                    py = pwide()
                    for c0, c1 in bank_chunks(D):
                        for di in range(ND):
                            nc.tensor.matmul(
                                py[:, c0:c1],
                                att3[:, di, :],
                                wp[di][:, c0:c1],
                                start=(di == 0), stop=(di == ND - 1),
                            )
                    nc.vector.tensor_tensor(
                        out=y_t[:], in0=py[:, :D], in1=b_bcast[:],
                        op=mybir.AluOpType.add,
                    )
                    # SWDGE store keeps the (wait-blocking) HWDGE
                    # sequencers free for the transpose stream
                    nc.gpsimd.dma_start(
                        out_d[qi * 128:(qi + 1) * 128, :], y_t[:]
                    )

                # software pipeline: each block's AV runs LAG_AV blocks after
                # its XBAR transpose was launched (hiding the ~2.5us DMA
                # latency behind other blocks' scores/exp), and its normalize
                # a further LAG_FIN-LAG_AV blocks later (so DVE/ACT queue
                # heads never wait on fresh PE results). The last head's
                # finished token chunks go through the same lagging for their
                # projection. QK chunk pairs for the next head pair are issued
                # mid-head to spread PSUM-ring pressure.
                LAG_AV = 6
                LAG_FIN = 6
                front_q = []
                av_q = []
                proj_q = []

                def drain(front_limit, av_limit):
                    while len(front_q) > front_limit:
                        h, qi, wT = front_q.pop(0)
                        av_q.append((h, qi, attn_av(h, qi, wT)))
                    while len(av_q) > av_limit:
                        h, qi, po = av_q.pop(0)
                        attn_fin(h, qi, po)
                        if h == H - 1:
                            proj_q.append((qi, proj_front(qi)))
                        if len(proj_q) > 2:
                            proj_back(*proj_q.pop(0))

                qk_chunk(0)
                qk_chunk(6)
                for h in range(H):
                    r = h // 2
                    for qi in range(NT):
                        front_q.append((h, qi, attn_front(h, qi)))
                        drain(LAG_AV, LAG_FIN - LAG_AV)
                        if h % 2 == 1 and r < 5:
                            if qi == 2:
                                qk_chunk(r + 1)
                            elif qi == 5:
                                qk_chunk(7 + r)
                drain(0, 0)
                while proj_q:
                    proj_back(*proj_q.pop(0))

    nc.compile()
    return nc


_NC_CACHE = None


def _get_nc():
    global _NC_CACHE
    if _NC_CACHE is None:
        nc = bacc.Bacc(
            "TRN2",
            target_bir_lowering=False,
            debug=False,
            num_devices=N_CORES,
        )
        build_mha(nc)
        _NC_CACHE = nc
    return _NC_CACHE


def kernel(x, W_qkv, W_proj, b_proj):
    nc = _get_nc()
    x = np.ascontiguousarray(np.asarray(x, dtype=np.float32))
    W_qkv = np.ascontiguousarray(np.asarray(W_qkv, dtype=np.float32))
    W_proj = np.ascontiguousarray(np.asarray(W_proj, dtype=np.float32))
    b_proj = np.ascontiguousarray(
        np.asarray(b_proj, dtype=np.float32).reshape(1, D)
    )
    in_maps = [
        {"x": x[b], "W_qkv": W_qkv, "W_proj": W_proj, "b_proj": b_proj}
        for b in range(N_CORES)
    ]
    res = run_bass_kernel_spmd(nc, in_maps, core_ids=list(range(N_CORES)))
    return np.stack([res.results[b]["out"] for b in range(N_CORES)], axis=0)
